# revision 40
# baseline (speedup 1.0000x reference)
"""Baichuan paged-attention layer on 8 trn2 cores, tensor-parallel over heads.

Per core c: heads 4c..4c+3. Device computes QKV proj, RoPE, attention vs
[gathered history KV + new KV], and a partial o_proj [T, HID] against
w_o[:, 512c:512c+512]. Host gathers history KV pages, builds RoPE tables,
and sums the 8 partial outputs (bf16 partials, f64 accumulate). All matmul
operands are bf16 (fp32 PSUM accumulation); softmax/RoPE arithmetic stays
fp32 on the vector/scalar engines.

The PE stream is kept at the pure-GEMM floor (~489us/core):
 - softmax denominators come from DVE-accumulated exp partials folded by a
   Pool partition_all_reduce (no ones-matmul, the PE never streams exp
   tiles twice);
 - causal masking is a 0/1 multiply on the exp tile (Pool, SBUF) since
   GPSIMD cannot touch PSUM and pre-exp adds would ride the DVE queue;
 - RoPE's rotate-half is two partition-shifted DVE multiplies reading the
   PSUM projection directly (sign folded into the sin table) -- no
   permutation matmul, no PSUM->SBUF staging copy;
 - attention runs head-PAIRS with PV trailing QK by 3 stage slots, and
   o_proj units of the previous sequence(s) drip into the stage slots as
   PE filler, so the QK->exp->PV cross-engine latency hides;
 - startup DMAs are split per-slice across the three DMA-capable queues so
   the first matmuls start ~2.7us in; out-DMA alternates sync/pool.
"""
import sys

sys.path.insert(0, "/opt/trn_rl_repo")
import numpy as np

H = 32; D = 128; HID = 4096; BS = 64; NBLOCKS = 128
B = 4; QLEN = 512; MAXBLK = 24; ROPE_BASE = 10000.0
T = B * QLEN; NCORES = 8; HC = H // NCORES; W = HC * D  # 4 heads, 512 wide
NEG = -1.0e30
SCALE = 1.0 / float(np.sqrt(D))

_cache = {}
last_results = None  # BassKernelResults of the most recent run (for test.py)

# pool sizing knobs
BUFS = dict(cs=2, hid=8, wq=2, wv=2, qkr=16, qs=2, ropet=2, vsb=8,
            kh=2, vh=2, exp=8, smol=1, stg=4, attn=9, wo=8)


def _round128(x):
    return (x + 127) // 128 * 128


def _np_bf16():
    from concourse import mybir
    return mybir.dt.np(mybir.dt.bfloat16)


def _build(hist):
    import concourse.bass as bass
    import concourse.tile as tile
    from concourse import bacc, bass_isa, mybir

    F32 = mybir.dt.float32
    BF16 = mybir.dt.bfloat16
    np_bf16 = _np_bf16()

    hv = [_round128(h) for h in hist]
    SH = [x // 128 for x in hv]

    nc = bacc.Bacc("TRN2", target_bir_lowering=False, debug=False,
                   num_devices=NCORES)
    hiddenT_d = nc.dram_tensor("hiddenT", [HID, T], BF16, kind="ExternalInput")
    # wql: [p, (rt*4+kc)*1024 + s*128 + c] = wqk[rt*128+c, kc*1024+s*128+p]
    wql_d = nc.dram_tensor("wql", [128, 8 * 4 * 1024], BF16,
                           kind="ExternalInput")
    wvT_d = nc.dram_tensor("wvT", [HID, W], BF16, kind="ExternalInput")
    woT_d = nc.dram_tensor("woT", [W, HID], BF16, kind="ExternalInput")
    kh_d = [nc.dram_tensor(f"khT{b}", [W, hv[b]], BF16, kind="ExternalInput")
            if hv[b] else None for b in range(B)]
    vh_d = [nc.dram_tensor(f"vh{b}", [hv[b], W], BF16, kind="ExternalInput")
            if hv[b] else None for b in range(B)]
    out_d = nc.dram_tensor("out", [T, HID], BF16, kind="ExternalOutput")

    # host-built tables baked into the NEFF
    inv = 1.0 / (ROPE_BASE ** (np.arange(0, D, 2) / D))
    pos = np.concatenate([h + np.arange(QLEN) for h in hist]).astype(np.float64)
    ang = np.concatenate([inv, inv])[:, None] * pos[None, :]
    cos_d = nc.inline_tensor(np.cos(ang).astype(np.float32), name="cosT")
    # sign-folded sin: rows 0..63 negated, so RoPE's rotate-half becomes two
    # partition-shifted DVE multiplies straight out of the PSUM projection
    sinm = np.sin(ang).astype(np.float32)
    sinm[:64] *= -1.0
    sin_d = nc.inline_tensor(sinm, name="sinT")

    # 0/1 multiplicative masks applied to exp-output tiles in SBUF (the
    # Pool engine cannot touch PSUM, so scores are not masked pre-exp)
    mask_np = np.where(np.arange(128)[:, None] <= np.arange(128)[None, :],
                       1.0, 0.0).astype(_np_bf16())
    mask_d = nc.inline_tensor(mask_np, name="maskS")

    pad_np = np.ones((128, B), np.float32)
    for b in range(B):
        if hv[b]:
            pad_np[:, b] = np.where(hv[b] - 128 + np.arange(128) >= hist[b],
                                    0.0, 1.0)
    pad_d = nc.inline_tensor(pad_np.astype(_np_bf16()), name="padc")

    from contextlib import ExitStack

    with tile.TileContext(nc) as tc:
        with ExitStack() as ctx:
            cpool = ctx.enter_context(tc.tile_pool(name="const", bufs=1))
            apool = ctx.enter_context(
                tc.tile_pool(name="attn", bufs=BUFS["attn"]))
            wopool = ctx.enter_context(
                tc.tile_pool(name="wop", bufs=BUFS["wo"]))
            pspool = ctx.enter_context(
                tc.tile_pool(name="psum", bufs=8, space="PSUM"))
            hid0pool = ctx.enter_context(
                tc.tile_pool(name="hid", bufs=BUFS["hid"]))
            wq0pool = ctx.enter_context(
                tc.tile_pool(name="wst", bufs=BUFS["wq"]))
            # critical-path prefetch: group-0 hidden (chunk-major) and the
            # first rt's wq tiles go to the FRONT of the DMA rings, spread
            # over four queues so the first matmuls start ~4us in.
            pre_hid = {0: [None] * 4, 1: [None] * 4}
            pre_wqt = {}

            def pre_hid_dma(kc, b, eng):
                tsl = slice(b * QLEN, (b + 1) * QLEN)
                ht = hid0pool.tile([128, 8, QLEN], BF16, tag="hid",
                                   name=f"prehid{b}_{kc}")
                eng.dma_start(
                    ht[:],
                    hiddenT_d[kc * 1024:(kc + 1) * 1024, tsl]
                    .rearrange("(s p) t -> p s t", p=128))
                pre_hid[b][kc] = ht
            # all rt0 weights first: the scheduler hoists LDWEIGHTS, so the
            # in-order PE stream blocks until the LAST rt0 weight tile lands
            # queue plan (3 DMA-capable queues):
            #   gpsimd: wq0, hid(kc1,b1), cos1/sin1, pt, mask, pad
            #   sync:   hid(kc0,b0), hid(kc1,b0), hid(kc2,b0), cos0/sin0,
            #           hid(kc3,b0)
            #   scalar: hid(kc0,b1), wq1, hid(kc2,b1), hid(kc3,b1)
            cspool = ctx.enter_context(tc.tile_pool(name="cs", bufs=BUFS["cs"]))

            def pre_cs_dma(b, eng):
                tsl = slice(b * QLEN, (b + 1) * QLEN)
                ct = cspool.tile([128, QLEN], F32, tag="cos", name=f"precos{b}")
                eng.dma_start(ct[:], cos_d[:, tsl])
                st = cspool.tile([128, QLEN], F32, tag="sin", name=f"presin{b}")
                eng.dma_start(st[:], sin_d[:, tsl])
                return (ct, st)

            # wq0 split in half tiles (sync + pool queues) so the first
            # Ldweights lands ~1us in; hidden chunks stream behind them.
            # b0's stream rides sync, b1's rides pool -- both queues keep
            # emission order; the Act queue (which the scheduler reorders)
            # gets only the less time-critical tables and wq1.
            pre_wq0 = []
            for kci in range(2):
                wqh = wq0pool.tile([128, 1024], BF16, tag="wqh", bufs=2,
                                   name=f"prewq0_{kci}")
                (nc.sync, nc.gpsimd)[kci].dma_start(
                    wqh[:], wql_d[:, kci * 1024:(kci + 1) * 1024])
                pre_wq0.append(wqh)
            # kc0 streams per-s slices so matmul s consumes while s+1 lands
            for b in (0, 1):
                tsl = slice(b * QLEN, (b + 1) * QLEN)
                ht = hid0pool.tile([128, 8, QLEN], BF16, tag="hid",
                                   name=f"prehid{b}_0")
                for s in range(8):
                    (nc.sync, nc.gpsimd)[b].dma_start(
                        ht[:, s, :],
                        hiddenT_d[s * 128:(s + 1) * 128, tsl])
                pre_hid[b][0] = ht
            pre_wqt[1] = wq0pool.tile([128, 2048], BF16, tag="wq",
                                      name="prewq1")
            nc.scalar.dma_start(pre_wqt[1][:], wql_d[:, 2048:4096])
            pre_hid_dma(1, 0, nc.sync)
            pre_hid_dma(1, 1, nc.gpsimd)
            pre_cs = {0: pre_cs_dma(0, nc.scalar)}
            pre_hid_dma(2, 0, nc.sync)
            pre_hid_dma(2, 1, nc.gpsimd)
            pre_cs[1] = pre_cs_dma(1, nc.scalar)
            pre_hid_dma(3, 0, nc.sync)
            pre_hid_dma(3, 1, nc.gpsimd)

            mask_t = cpool.tile([128, 128], BF16, tag="mask")
            nc.scalar.dma_start(mask_t[:], mask_d[:])
            pad_t = cpool.tile([128, B], BF16, tag="pad")
            nc.scalar.dma_start(pad_t[:], pad_d[:])

            wots = []

            hidpool = hid0pool
            wqpool = wq0pool
            wvpool = ctx.enter_context(
                tc.tile_pool(name="wvst", bufs=BUFS["wv"]))
            qkrpool = ctx.enter_context(
                tc.tile_pool(name="qkr", bufs=BUFS["qkr"]))
            rppool = ctx.enter_context(
                tc.tile_pool(name="rope", bufs=BUFS["qs"]))
            vpool = ctx.enter_context(
                tc.tile_pool(name="vsb", bufs=BUFS["vsb"]))
            khpool = ctx.enter_context(
                tc.tile_pool(name="khp", bufs=BUFS["kh"]))
            vhpool = ctx.enter_context(
                tc.tile_pool(name="vhp", bufs=BUFS["vh"]))
            epool = ctx.enter_context(
                tc.tile_pool(name="expp", bufs=BUFS["exp"]))
            smpool = ctx.enter_context(
                tc.tile_pool(name="smol", bufs=BUFS["smol"]))
            stpool = ctx.enter_context(
                tc.tile_pool(name="stg", bufs=BUFS["stg"]))
            # o_proj units drip into the attention stage slots of the NEXT
            # sequence(s) as PE filler: the mask->exp->PV chain latency per
            # stage is covered by one po unit (~0.84us of dense matmul).
            pending = []

            def fill_one():
                while pending:
                    try:
                        next(pending[0])
                        return
                    except StopIteration:
                        pending.pop(0)

            def drain():
                while pending:
                    try:
                        next(pending[0])
                    except StopIteration:
                        pending.pop(0)

            if True:
                for g in range(2):
                    bs = (2 * g, 2 * g + 1)
                    cos_t, sin_t, hid_c = {}, {}, {}
                    for b in bs:
                        tsl = slice(b * QLEN, (b + 1) * QLEN)
                        if g == 0:
                            cos_t[b], sin_t[b] = pre_cs[b]
                        else:
                            cos_t[b] = cspool.tile([128, QLEN], F32, tag="cos", name=f"cos{b}")
                            nc.scalar.dma_start(cos_t[b][:], cos_d[:, tsl])
                            sin_t[b] = cspool.tile([128, QLEN], F32, tag="sin", name=f"sin{b}")
                            nc.scalar.dma_start(sin_t[b][:], sin_d[:, tsl])
                        if g == 0:
                            hid_c[b] = pre_hid[b]
                        else:
                            hid_c[b] = []
                            for kc in range(4):
                                ht = hidpool.tile([128, 8, QLEN], BF16,
                                                  tag="hid",
                                                  name=f"hid{b}_{kc}")
                                nc.sync.dma_start(
                                    ht[:],
                                    hiddenT_d[kc * 1024:(kc + 1) * 1024, tsl]
                                    .rearrange("(s p) t -> p s t", p=128))
                                hid_c[b].append(ht)

                    # ---- QK proj (wq shared across the group) + RoPE.
                    # V-proj chunks interleave: seq bs[0] during rt 0-3,
                    # bs[1] during rt 4-7.
                    qk_rot = {b: [] for b in bs}
                    v_sb, v_ps = {}, {}
                    for b in bs:
                        v_sb[b] = [vpool.tile([128, W], BF16, tag="vsb",
                                              name=f"vsb{b}_{i}")
                                   for i in range(4)]
                    for rt in range(8):
                        if rt == 0 or rt == 4:
                            vb = bs[rt // 4]
                            v_ps[vb] = [pspool.tile([128, W], F32, tag="ps",
                                                    name=f"vps{vb}_{i}")
                                        for i in range(4)]
                        pq = {}
                        for b in bs:
                            pq[b] = pspool.tile([128, QLEN], F32, tag="ps", name=f"pq{b}")
                        for kcp in range(2):
                            if g == 0 and rt == 0 and kcp == 0:
                                wqt = None
                            elif g == 0 and rt == 0:
                                wqt = pre_wqt[kcp]
                            else:
                                wqt = wqpool.tile([128, 2048], BF16,
                                                  tag="wq")
                                nc.sync.dma_start(
                                    wqt[:],
                                    wql_d[:, (rt * 2 + kcp) * 2048:
                                          (rt * 2 + kcp + 1) * 2048])
                            for kci in range(2):
                                kc = kcp * 2 + kci
                                if wqt is None:
                                    wsl = pre_wq0[kci][:, 0:1024]
                                else:
                                    wsl = wqt[:, kci * 1024:(kci + 1) * 1024]
                                for s in range(8):
                                    for b in bs:
                                        nc.tensor.matmul(
                                            pq[b][:],
                                            wsl[:, s * 128:(s + 1) * 128],
                                            hid_c[b][kc][:, s, :],
                                            start=(kc == 0 and s == 0),
                                            stop=(kc == 3 and s == 7))
                        for b in bs:
                            t1 = rppool.tile([128, QLEN], F32, tag="t1",
                                             bufs=BUFS["ropet"])
                            nc.vector.tensor_mul(t1[0:64, :],
                                                 pq[b][64:128, :],
                                                 sin_t[b][0:64, :])
                            nc.vector.tensor_mul(t1[64:128, :],
                                                 pq[b][0:64, :],
                                                 sin_t[b][64:128, :])
                            t2 = rppool.tile([128, QLEN], F32, tag="t2",
                                             bufs=BUFS["ropet"])
                            nc.vector.tensor_mul(t2[:], pq[b][:], cos_t[b][:])
                            qr = qkrpool.tile([128, QLEN], BF16, tag="qkr")
                            nc.vector.tensor_add(qr[:], t1[:], t2[:])
                            qk_rot[b].append(qr)
                        vb = bs[rt // 4]
                        rr = rt % 4
                        if g == 0 and rt >= 4:
                            continue  # b1's V-proj is deferred as filler
                        for kc2 in range(2 * rr, 2 * rr + 2):
                            wvt = wvpool.tile([128, 4, W], BF16, tag="wv")
                            nc.sync.dma_start(
                                wvt[:],
                                wvT_d[kc2 * 512:(kc2 + 1) * 512, :]
                                .rearrange("(s p) c -> p s c", p=128))
                            for s2 in range(4):
                                k = kc2 * 4 + s2
                                for tt in range(4):
                                    nc.tensor.matmul(
                                        v_ps[vb][tt][:],
                                        hid_c[vb][k // 8][:, k % 8,
                                                          tt * 128:(tt + 1) * 128],
                                        wvt[:, s2, :],
                                        start=(k == 0), stop=(k == 31))
                        if rr == 3:
                            for tt in range(4):
                                nc.vector.tensor_copy(v_sb[vb][tt][:],
                                                      v_ps[vb][tt][:])

                    def vproj_gen(vb):
                        # deferred V-projection: fills attn(b0) stage slots
                        # in group 0, where no o_proj filler exists yet
                        v_ps[vb] = [pspool.tile([128, W], F32, tag="ps",
                                                name=f"vpsd{vb}_{i}")
                                    for i in range(4)]
                        for kc2 in range(8):
                            wvt = wvpool.tile([128, 4, W], BF16, tag="wv",
                                              name="wvtd")
                            nc.sync.dma_start(
                                wvt[:],
                                wvT_d[kc2 * 512:(kc2 + 1) * 512, :]
                                .rearrange("(s p) c -> p s c", p=128))
                            for s2 in range(4):
                                k = kc2 * 4 + s2
                                for tt in range(4):
                                    nc.tensor.matmul(
                                        v_ps[vb][tt][:],
                                        hid_c[vb][k // 8][:, k % 8,
                                                          tt * 128:
                                                          (tt + 1) * 128],
                                        wvt[:, s2, :],
                                        start=(k == 0), stop=(k == 31))
                                yield
                        for tt in range(4):
                            nc.vector.tensor_copy(v_sb[vb][tt][:],
                                                  v_ps[vb][tt][:])

                    if g == 0:
                        pending.append(vproj_gen(bs[1]))

                    # ---- attention + o_proj partial per sequence; seq b1's
                    # first head-pair is emitted before o_proj(b0) so the
                    # last at-normalization chain of each seq hides under
                    # other PE work.
                    vht_m, attn_m = {}, {}
                    for b in bs:
                        vht_m[b] = None
                        attn_m[b] = []
                        if SH[b]:
                            vht_m[b] = vhpool.tile([128, 8, W], BF16,
                                                   tag="vh", name=f"vh_t{b}")
                            nc.sync.dma_start(
                                vht_m[b][:, :SH[b], :],
                                vh_d[b][:].rearrange("(s p) c -> p s c",
                                                     p=128))
                        if g == 0 and b == bs[0]:
                            # o_proj weights: after the startup window AND
                            # after the first attention's history-V tile
                            # (needed ~25us before o_proj(b0) consumes them)
                            for ic in range(8):
                                isl = slice(ic * 512, (ic + 1) * 512)
                                wot = wopool.tile([128, 4, 512], BF16,
                                                  tag="wo", name=f"wot{ic}")
                                nc.scalar.dma_start(
                                    wot[:],
                                    woT_d[:, isl]
                                    .rearrange("(s p) c -> p s c", p=128))
                                wots.append(wot)

                    def attn_pair(b, hp):
                        vht = vht_m[b]
                        attn_b = attn_m[b]
                        S = SH[b] + 4
                        # heads run in interleaved pairs: head h1's QK fills
                        # the PE wait while head h0's mask->exp chain runs,
                        # and PV trails its own QK by one stage slot.
                        if True:
                            pair = (2 * hp, 2 * hp + 1)
                            kh_t, pv, exs, prev = {}, {}, {}, {}
                            for h in pair:
                                if SH[b]:
                                    kh_t[h] = khpool.tile(
                                        [128, hv[b]], BF16, tag="kh",
                                        name=f"kh{b}_{h}")
                                    nc.sync.dma_start(
                                        kh_t[h][:],
                                        kh_d[b][h * 128:(h + 1) * 128, :])
                                pv[h] = pspool.tile([128, QLEN], F32,
                                                    tag="ps", name=f"pv{h}")
                                # partial kv-sums of exp accumulate on DVE;
                                # Pool folds partitions at the end, so the PE
                                # never streams ex twice (no ones-matmul).
                                exs[h] = smpool.tile([128, QLEN], F32,
                                                     tag="exs", bufs=2,
                                                     name=f"exs{b}_{h}")
                            LAG = 3
                            for st in range(S + LAG):
                                fill_one()
                                for h in pair:
                                    if st >= LAG:
                                        exp_, vtp, offp, wdtp, stp = \
                                            prev[h].pop(0)
                                        nc.tensor.matmul(
                                            pv[h][:, offp:], vtp,
                                            exp_[:, :wdtp],
                                            start=(stp == 0),
                                            stop=(stp == S - 1))
                                    if st < S:
                                        sc = pspool.tile([128, QLEN], F32,
                                                         tag="ps", name="sc")
                                        if st < SH[b]:
                                            off, wdt = 0, QLEN
                                            lhsT = kh_t[h][:, st * 128:
                                                           (st + 1) * 128]
                                            rhs = qk_rot[b][h][:]
                                        else:
                                            # new-K stage j: queries t < 128j
                                            # are fully masked -- skip them
                                            j = st - SH[b]
                                            off, wdt = 128 * j, QLEN - 128 * j
                                            lhsT = qk_rot[b][4 + h][
                                                :, j * 128:(j + 1) * 128]
                                            rhs = qk_rot[b][h][:, off:]
                                        nc.tensor.matmul(sc[:, :wdt], lhsT,
                                                         rhs, start=True,
                                                         stop=True)
                                        ex = epool.tile([128, QLEN], BF16,
                                                        tag="exp")
                                        nc.scalar.activation(
                                            ex[:, :wdt], sc[:, :wdt],
                                            mybir.ActivationFunctionType.Exp,
                                            scale=SCALE)
                                        if (st == SH[b] - 1
                                                and hist[b] != hv[b]):
                                            nc.gpsimd.tensor_scalar_mul(
                                                ex[:, :wdt], ex[:, :wdt],
                                                pad_t[:, b:b + 1])
                                        if st >= SH[b]:
                                            # zero the masked upper triangle
                                            # of the leading diagonal block
                                            # (post-exp, SBUF, on Pool)
                                            nc.gpsimd.tensor_mul(
                                                ex[:, :128], ex[:, :128],
                                                mask_t[:])
                                        if st == 0:
                                            nc.vector.tensor_copy(exs[h][:],
                                                                  ex[:])
                                        else:
                                            nc.vector.tensor_add(
                                                exs[h][:, off:],
                                                exs[h][:, off:],
                                                ex[:, :wdt])
                                        if st < SH[b]:
                                            vt = vht[:, st,
                                                     h * 128:(h + 1) * 128]
                                        else:
                                            vt = v_sb[b][st - SH[b]][
                                                :, h * 128:(h + 1) * 128]
                                        prev.setdefault(h, []).append(
                                            (ex, vt, off, wdt, st))
                            for h in pair:
                                dnb = smpool.tile([128, QLEN], F32,
                                                  tag="dnb",
                                                  name=f"dnb{b}_{h}")
                                nc.gpsimd.partition_all_reduce(
                                    dnb[:], exs[h][:], channels=128,
                                    reduce_op=bass_isa.ReduceOp.add)
                                rc = smpool.tile([128, QLEN], F32, tag="rc",
                                                 name=f"rc{b}_{h}")
                                nc.vector.reciprocal(rc[:], dnb[:])
                                at = apool.tile([128, QLEN], BF16,
                                                tag="attn")
                                nc.vector.tensor_mul(at[:], pv[h][:], rc[:])
                                attn_b.append(at)

                    def oproj(b, attn_b):
                        # o_proj partial for this sequence's 4 token tiles,
                        # one (ic, q) unit per yield
                        for ic in range(8):
                            isl = slice(ic * 512, (ic + 1) * 512)
                            for q in range(4):
                                tt = b * 4 + q
                                po = pspool.tile([128, 512], F32, tag="ps")
                                for jt in range(4):
                                    nc.tensor.matmul(
                                        po[:],
                                        attn_b[jt][:, q * 128:(q + 1) * 128],
                                        wots[ic][:, jt, :],
                                        start=(jt == 0), stop=(jt == 3))
                                st_ = stpool.tile([128, 512], BF16, tag="stg")
                                if (ic + q) % 2 == 0:
                                    nc.vector.tensor_copy(st_[:], po[:])
                                else:
                                    nc.scalar.copy(st_[:], po[:])
                                ((nc.sync, nc.gpsimd)[(ic + q) % 2]
                                 ).dma_start(
                                    out_d[tt * 128:(tt + 1) * 128, isl],
                                    st_[:])
                                yield

                    # b0's pairs drain the previous group's spilled o_proj;
                    # b1's pairs drain o_proj(b0); o_proj(b1) spills forward
                    # (final group drains everything at the end).
                    attn_pair(bs[0], 0)
                    attn_pair(bs[0], 1)
                    drain()
                    pending.append(oproj(bs[0], attn_m[bs[0]]))
                    attn_pair(bs[1], 0)
                    attn_pair(bs[1], 1)
                    drain()
                    pending.append(oproj(bs[1], attn_m[bs[1]]))
                drain()
    nc.compile()
    return {"nc": nc}


def _get(hist):
    if hist not in _cache:
        _cache[hist] = _build(hist)
    return _cache[hist]


def prepare_in_maps(inputs):
    np_bf16 = _np_bf16()
    hidden = np.asarray(inputs["hidden_states"], np.float32)
    w_pack = np.asarray(inputs["w_pack"], np.float32)
    w_o = np.asarray(inputs["w_o"], np.float32)
    kc = np.asarray(inputs["key_cache"], np.float32).reshape(NBLOCKS * BS, H, D)
    vc = np.asarray(inputs["value_cache"], np.float32).reshape(NBLOCKS * BS, H, D)
    bo = np.asarray(inputs["block_offsets"], np.int32)
    hist = tuple(int(x) for x in np.asarray(inputs["history_lengths"]))
    assert all(0 <= h and h + QLEN <= MAXBLK * BS for h in hist)
    hv = [_round128(h) for h in hist]

    built = _get(hist)
    hiddenT = np.ascontiguousarray(hidden.T).astype(np_bf16)

    in_maps = []
    for c in range(NCORES):
        rs = slice(c * W, (c + 1) * W)
        wqk = np.concatenate(
            [w_pack[rs], w_pack[HID + c * W:HID + (c + 1) * W]], axis=0)
        # wql[p, rt, kc, s, c] = wqk[rt*128+c, kc*1024+s*128+p]
        wql = np.ascontiguousarray(
            wqk.reshape(8, 128, 4, 8, 128).transpose(4, 0, 2, 3, 1)
            .reshape(128, 8 * 4 * 1024)).astype(np_bf16)
        wv = w_pack[2 * HID + c * W:2 * HID + (c + 1) * W]
        im = {
            "hiddenT": hiddenT,
            "wql": wql,
            "wvT": np.ascontiguousarray(wv.T).astype(np_bf16),
            "woT": np.ascontiguousarray(w_o[:, rs].T).astype(np_bf16),
        }
        for b in range(B):
            if not hv[b]:
                continue
            nblk = (hist[b] + BS - 1) // BS
            rows = (bo[b, :nblk, None] * BS +
                    np.arange(BS)[None, :]).reshape(-1)[:hist[b]]
            khp = np.zeros((hv[b], HC, D), np.float32)
            khp[:hist[b]] = kc[rows][:, c * HC:(c + 1) * HC, :]
            vhp = np.zeros((hv[b], HC, D), np.float32)
            vhp[:hist[b]] = vc[rows][:, c * HC:(c + 1) * HC, :]
            im[f"khT{b}"] = np.ascontiguousarray(
                khp.transpose(1, 2, 0).reshape(W, hv[b])).astype(np_bf16)
            im[f"vh{b}"] = np.ascontiguousarray(
                vhp.reshape(hv[b], W)).astype(np_bf16)
        in_maps.append(im)
    return built["nc"], in_maps


def kernel(**inputs):
    global last_results
    from concourse.bass_utils import run_bass_kernel_spmd

    nc, in_maps = prepare_in_maps(inputs)
    last_results = run_bass_kernel_spmd(nc, in_maps,
                                        core_ids=list(range(NCORES)))
    acc = np.zeros((T, HID), np.float64)
    for c in range(NCORES):
        acc += last_results.results[c]["out"].astype(np.float64)
    return acc.astype(np.float32)

